# revision 22
# baseline (speedup 1.0000x reference)
"""MoE BERT head (soft routing) on 8 Trainium2 NeuronCores.

Reference computation (B=4096, H=1024, E=32, O=1024):
    gate = softmax(X @ gate_W + gate_b, axis=experts)            [B, E]
    h_e  = relu(LN(X @ W1[e] + b1[e]) * ln_g[e] + ln_b[e])       [B, H] per expert
    out  = sum_e gate[:, e] * (h_e @ W2[e] + b2[e])              [B, O]

Strategy: expert-parallel over 8 cores (4 experts/core), two passes of 2
experts per core.  Per pass, batch is processed in column-chunks of 1024;
for each chunk both experts' first GEMM + LayerNorm run in the natural
[batch, feature] layout (bn_stats), the gating weight (softmax, computed
on-device from the full gate matrix) is folded into the LN affine as a
per-row scale, activations are PE-transposed into [feature, batch] layout
and the second GEMM accumulates BOTH experts in PSUM (K-concatenation,
K=2048).  Each core returns per-pass partial outputs out.T [O, B]; the
host sums partials over cores/passes and transposes.  The per-expert
output bias enters the total as gate @ b2, added on the host (exact).

All matmuls run as float32r (fp32 storage, ~fp22 multiply, fp32
accumulate) - full PE rate with ~1e-4 relative accuracy.
"""

import os
import sys
from contextlib import ExitStack

for _p in ("/opt/trn_rl_repo", "/root/.axon_site/_ro/trn_rl_repo"):
    if os.path.isdir(_p) and _p not in sys.path:
        sys.path.insert(0, _p)

import numpy as np

import concourse.bass as bass
import concourse.mybir as mybir
import concourse.tile as tile
from concourse import bacc
from concourse.bass_utils import run_bass_kernel_spmd
from concourse.masks import make_identity

B, H, E, O = 4096, 1024, 32, 1024
LN_EPS = 1e-5
N_CORES = 8
E_PER_CORE = E // N_CORES            # 4
N_PASSES = 2
E_PER_PASS = E_PER_CORE // N_PASSES  # 2
B_CHUNK = 1024
N_CHUNKS = B // B_CHUNK              # 4
P = 128
KT = H // P                          # 8 k-tiles over the hidden dim
BT = B_CHUNK // P                    # 8 batch tiles per chunk
F32 = mybir.dt.float32
F32R = mybir.dt.float32r

Relu = mybir.ActivationFunctionType.Relu
Exp = mybir.ActivationFunctionType.Exp
Sqrt = mybir.ActivationFunctionType.Sqrt
Alu = mybir.AluOpType
AxX = mybir.AxisListType.X

_CACHE = {}
_LAST_IN_MAPS = None

# scheduling knobs (pool depths / grouping); tuned via TimelineSim sweep
_CFG = {
    "xtp": 9, "w1p": 14, "w2p": 2, "hscp": 9, "hstp": 2, "osbp": 3,
    "hps": 3, "tps": 2, "ops": 2, "gps": 1,
    "btg": 2,       # batch tiles per GEMM1 psum group
    "stats_sbuf": 0,  # bn_stats reads SBUF copy instead of PSUM
    "cb_alt": 0,      # alternate transpose copybacks between ACT and DVE
}



class _K:
    """Holds program-build state (pools, dram handles, flags)."""


def _load_chunk_xt(k_, c0):
    nc = k_.nc
    xts = []
    for k in range(KT):
        t = k_.xtp.tile([P, B_CHUNK], F32R, tag="xt")
        nc.sync.dma_start(out=t[:], in_=k_.xt_d[k * P:(k + 1) * P, c0:c0 + B_CHUNK])
        xts.append(t)
    return xts


def _gate_softmax(k_, xts):
    """gate = softmax(X @ gate_W + gate_b) for all E; returns gsb [P, BT, E]."""
    nc = k_.nc
    g_ps = k_.gps.tile([P, BT, E], F32)
    for bt in range(BT):
        for k in range(KT):
            nc.tensor.matmul(
                g_ps[:, bt, :],
                xts[k][:, bt * P:(bt + 1) * P],
                k_.gw_sb[:, k, :],
                start=(k == 0), stop=(k == KT - 1))
    gsb = k_.gselp.tile([P, BT, E], F32, tag="gsb")
    nc.scalar.copy(gsb[:], g_ps[:])
    if k_.use_gb:
        for bt in range(BT):
            nc.vector.tensor_add(gsb[:, bt, :], gsb[:, bt, :], k_.gb_bc[:])
    gmax = k_.smallp.tile([P, BT], F32, tag="gmax")
    nc.vector.tensor_reduce(gmax[:], gsb[:], axis=AxX, op=Alu.max)
    nc.vector.tensor_scalar_mul(gmax[:], gmax[:], -1.0)
    for bt in range(BT):
        nc.scalar.activation(gsb[:, bt, :], gsb[:, bt, :], Exp,
                             bias=gmax[:, bt:bt + 1])
    gsum = k_.smallp.tile([P, BT], F32, tag="gsum")
    nc.vector.tensor_reduce(gsum[:], gsb[:], axis=AxX, op=Alu.add)
    nc.vector.reciprocal(gsum[:], gsum[:])
    for bt in range(BT):
        nc.vector.tensor_scalar_mul(gsb[:, bt, :], gsb[:, bt, :],
                                    gsum[:, bt:bt + 1])
    return gsb


def _expert_gemm1_ln(k_, xts, gsb, e):
    """GEMM1 + LayerNorm + gating fold + PE transpose for local expert e.

    Returns hsT_e [P, KT, B_CHUNK] fp32r in [feature, batch] layout,
    already relu'd and scaled by the gating weight.
    """
    nc = k_.nc
    b1_bc = lng_bc = lnb_bc = None
    if k_.use_b1:
        b1_bc = k_.bcp.tile([P, H], F32, tag="b1bc")
        nc.gpsimd.dma_start(out=b1_bc[:], in_=k_.b1_d[e].partition_broadcast(P))
    if k_.use_lng:
        lng_bc = k_.bcp.tile([P, H], F32, tag="lngbc")
        nc.gpsimd.dma_start(out=lng_bc[:], in_=k_.lng_d[e].partition_broadcast(P))
    if k_.use_lnb:
        lnb_bc = k_.bcp.tile([P, H], F32, tag="lnbbc")
        nc.gpsimd.dma_start(out=lnb_bc[:], in_=k_.lnb_d[e].partition_broadcast(P))

    # ---- GEMM1: h = X @ W1[e]  ([batch, feature] layout)
    h_sc = [k_.hscp.tile([P, H], F32R, tag="hsc", name=f"hsc{i}") for i in range(BT)]
    stats = [k_.smallp.tile([P, 2, 6], F32, tag="stats", name=f"stats{i}") for i in range(BT)]
    G = _CFG["btg"]
    for dch in range(2):
        w1s = {}
        for k in range(KT):
            t = k_.w1p.tile([P, 512], F32R, tag="w1", name=f"w1_{k}")
            nc.sync.dma_start(
                out=t[:],
                in_=k_.w1_d[e, k * P:(k + 1) * P, dch * 512:(dch + 1) * 512])
            w1s[(k, dch)] = t
        for btg in range(BT // G):
            hpg = [k_.hps.tile([P, 512], F32, tag="hps", name=f"hps{i}") for i in range(G)]
            for k in range(KT):
                for b2i in range(G):
                    bt = btg * G + b2i
                    nc.tensor.matmul(
                        hpg[b2i][:],
                        xts[k][:, bt * P:(bt + 1) * P],
                        w1s[(k, dch)][:],
                        start=(k == 0), stop=(k == KT - 1))
            for b2i in range(G):
                bt = btg * G + b2i
                dst = h_sc[bt][:, dch * 512:(dch + 1) * 512]
                if k_.use_b1:
                    nc.vector.tensor_add(
                        dst, hpg[b2i][:], b1_bc[:, dch * 512:(dch + 1) * 512])
                    nc.vector.bn_stats(stats[bt][:, dch, :], dst)
                elif _CFG["stats_sbuf"]:
                    nc.scalar.copy(dst, hpg[b2i][:])
                    nc.vector.bn_stats(stats[bt][:, dch, :], dst)
                else:
                    nc.vector.bn_stats(stats[bt][:, dch, :], hpg[b2i][:])
                    nc.scalar.copy(dst, hpg[b2i][:])

    # ---- LayerNorm + gating fold (all per-row scalars in this layout)
    for bt in range(BT):
        mv = k_.smallp.tile([P, 2], F32, tag="mv")
        nc.vector.bn_aggr(mv[:], stats[bt][:])
        rg = k_.smallp.tile([P, 1], F32, tag="rg")
        nc.scalar.activation(rg[:], mv[:, 1:2], Sqrt, bias=k_.eps_t[:])
        nc.vector.reciprocal(rg[:], rg[:])
        nc.vector.tensor_mul(rg[:], rg[:], gsb[:, bt, e:e + 1])
        # h = (h - mean) * (rstd * gate)
        nc.vector.tensor_scalar(
            h_sc[bt][:], h_sc[bt][:], mv[:, 0:1], rg[:],
            op0=Alu.subtract, op1=Alu.mult)
        if k_.use_lng:
            nc.vector.tensor_mul(h_sc[bt][:], h_sc[bt][:], lng_bc[:])
        if k_.use_lnb:
            # h += ln_b * gate   (gate>0 commutes with the later relu)
            nc.vector.scalar_tensor_tensor(
                h_sc[bt][:], lnb_bc[:], gsb[:, bt, e:e + 1], h_sc[bt][:],
                op0=Alu.mult, op1=Alu.add)

    # ---- PE transpose -> relu -> [feature, batch] fp32r
    hsT_e = k_.hstp.tile([P, KT, B_CHUNK], F32R, tag="hsT")
    for k in range(KT):
        for half in range(BT // 4):
            tp = k_.tps.tile([P, 512], F32R, tag="tps")
            for q in range(4):
                bt = half * 4 + q
                nc.tensor.transpose(
                    tp[:, q * P:(q + 1) * P],
                    h_sc[bt][:, k * P:(k + 1) * P],
                    k_.ident[:])
            cb_dst = hsT_e[:, k, half * 512:(half + 1) * 512]
            if _CFG["cb_alt"] and (k % 2 == 1):
                nc.vector.tensor_scalar_max(cb_dst, tp[:], 0.0)
            else:
                nc.scalar.activation(cb_dst, tp[:], Relu)
    return hsT_e


def _gemm2(k_, hsT, p_i, c0):
    """out.T[p_i] += sum over both experts: W2[e].T @ hsT[e] (PSUM K-concat)."""
    nc = k_.nc
    nke = E_PER_PASS * KT
    for ot in range(O // P):
        w2sb = k_.w2p.tile([P, nke, P], F32R, tag="w2")
        nc.sync.dma_start(out=w2sb[:], in_=k_.w2_d[p_i, ot])
        for bs in range(B_CHUNK // 512):
            op_t = k_.ops.tile([P, 512], F32, tag="ops")
            for ke in range(nke):
                nc.tensor.matmul(
                    op_t[:],
                    w2sb[:, ke, :],
                    hsT[ke // KT][:, ke % KT, bs * 512:(bs + 1) * 512],
                    start=(ke == 0), stop=(ke == nke - 1))
            osb = k_.osbp.tile([P, 512], F32, tag="osb")
            nc.scalar.copy(osb[:], op_t[:])
            nc.sync.dma_start(
                out=k_.outp_d[p_i, ot * P:(ot + 1) * P,
                              c0 + bs * 512:c0 + (bs + 1) * 512],
                in_=osb[:])


def _build_program(use_gb, use_b1, use_lng, use_lnb):
    nc = bacc.Bacc("TRN2", target_bir_lowering=False, debug=False,
                   num_devices=N_CORES)
    k_ = _K()
    k_.nc = nc
    k_.use_gb, k_.use_b1, k_.use_lng, k_.use_lnb = use_gb, use_b1, use_lng, use_lnb

    k_.xt_d = nc.dram_tensor("xt", [H, B], F32R, kind="ExternalInput")
    k_.w1_d = nc.dram_tensor("w1", [E_PER_CORE, H, H], F32R, kind="ExternalInput")
    k_.w2_d = nc.dram_tensor("w2t", [N_PASSES, O // P, P, E_PER_PASS * KT, P],
                             F32R, kind="ExternalInput")
    k_.gw_d = nc.dram_tensor("gw", [P, KT, E], F32R, kind="ExternalInput")
    k_.gb_d = nc.dram_tensor("gb", [E], F32, kind="ExternalInput") if use_gb else None
    k_.b1_d = (nc.dram_tensor("b1", [E_PER_CORE, H], F32, kind="ExternalInput")
               if use_b1 else None)
    k_.lng_d = (nc.dram_tensor("lng", [E_PER_CORE, H], F32, kind="ExternalInput")
                if use_lng else None)
    k_.lnb_d = (nc.dram_tensor("lnb", [E_PER_CORE, H], F32, kind="ExternalInput")
                if use_lnb else None)
    k_.outp_d = nc.dram_tensor("outp", [N_PASSES, O, B], F32,
                               kind="ExternalOutput")

    any_bcast = use_b1 or use_lng or use_lnb
    w1p_bufs = 10 if any_bcast else _CFG["w1p"]
    with tile.TileContext(nc) as tc, ExitStack() as ctx:
        pool = lambda name, bufs, **kw: ctx.enter_context(
            tc.tile_pool(name=name, bufs=bufs, **kw))
        singles = pool("singles", 1)
        k_.xtp = pool("xtp", _CFG["xtp"])
        k_.w1p = pool("w1p", w1p_bufs)
        k_.w2p = pool("w2p", _CFG["w2p"])
        k_.hscp = pool("hscp", _CFG["hscp"])
        k_.hstp = pool("hstp", _CFG["hstp"])
        k_.osbp = pool("osbp", _CFG["osbp"])
        k_.smallp = pool("smallp", 20)
        k_.gselp = pool("gselp", 2)
        k_.bcp = pool("bcp", 1)
        k_.hps = pool("hps", _CFG["hps"], space="PSUM")
        k_.tps = pool("tps", _CFG["tps"], space="PSUM")
        k_.ops = pool("ops", _CFG["ops"], space="PSUM")
        k_.gps = pool("gps", _CFG["gps"], space="PSUM")

        ident_f32 = singles.tile([P, P], F32)
        make_identity(nc, ident_f32)
        k_.ident = singles.tile([P, P], F32R)
        nc.vector.tensor_copy(k_.ident[:], ident_f32[:])
        k_.eps_t = singles.tile([P, 1], F32)
        nc.vector.memset(k_.eps_t, LN_EPS)
        k_.gw_sb = singles.tile([P, KT, E], F32R)
        nc.sync.dma_start(out=k_.gw_sb[:], in_=k_.gw_d[:])
        if use_gb:
            k_.gb_bc = singles.tile([P, E], F32)
            nc.gpsimd.dma_start(out=k_.gb_bc[:],
                                in_=k_.gb_d[:].partition_broadcast(P))

        for ci in range(N_CHUNKS):
            c0 = ci * B_CHUNK
            xts = _load_chunk_xt(k_, c0)
            gsb = _gate_softmax(k_, xts)
            for p_i in range(N_PASSES):
                hsT = []
                for e01 in range(E_PER_PASS):
                    e = E_PER_PASS * p_i + e01  # local expert idx, pass-major
                    hsT.append(_expert_gemm1_ln(k_, xts, gsb, e))
                _gemm2(k_, hsT, p_i, c0)

    nc.compile()
    return nc


def kernel(pooled_output, gate_W, gate_b, W1, b1, ln_g, ln_b, W2, b2):
    X = np.asarray(pooled_output, dtype=np.float32)
    gate_W = np.asarray(gate_W, dtype=np.float32)
    gate_b = np.asarray(gate_b, dtype=np.float32)
    W1 = np.asarray(W1, dtype=np.float32)
    b1 = np.asarray(b1, dtype=np.float32)
    ln_g = np.asarray(ln_g, dtype=np.float32)
    ln_b = np.asarray(ln_b, dtype=np.float32)
    W2 = np.asarray(W2, dtype=np.float32)
    b2 = np.asarray(b2, dtype=np.float32)

    use_gb = bool(np.any(gate_b != 0.0))
    use_b1 = bool(np.any(b1 != 0.0))
    use_lng = bool(np.any(ln_g != 1.0))
    use_lnb = bool(np.any(ln_b != 0.0))

    key = (use_gb, use_b1, use_lng, use_lnb)
    if key not in _CACHE:
        _CACHE[key] = _build_program(*key)
    nc = _CACHE[key]

    XT = np.ascontiguousarray(X.T)  # [H, B]

    in_maps = []
    for c in range(N_CORES):
        own = list(range(E_PER_CORE * c, E_PER_CORE * (c + 1)))
        rest = [e for e in range(E) if e not in own]
        perm = own + rest
        w1_c = np.ascontiguousarray(W1[own])
        # W2 tiled as [pass, o_tile, (e01, kd), 128, 128]
        w2_c = W2[own].reshape(N_PASSES, E_PER_PASS, KT, P, O // P, P)
        w2_c = np.ascontiguousarray(w2_c.transpose(0, 4, 3, 1, 2, 5))
        w2_c = w2_c.reshape(N_PASSES, O // P, P, E_PER_PASS * KT, P)
        m = {
            "xt": XT,
            "w1": w1_c,
            "w2t": w2_c,
            "gw": np.ascontiguousarray(
                gate_W[:, perm].reshape(KT, P, E).transpose(1, 0, 2)),
        }
        if use_gb:
            m["gb"] = np.ascontiguousarray(gate_b[perm])
        if use_b1:
            m["b1"] = np.ascontiguousarray(b1[own])
        if use_lng:
            m["lng"] = np.ascontiguousarray(ln_g[own])
        if use_lnb:
            m["lnb"] = np.ascontiguousarray(ln_b[own])
        in_maps.append(m)

    global _LAST_IN_MAPS
    _LAST_IN_MAPS = in_maps
    res = run_bass_kernel_spmd(nc, in_maps, core_ids=list(range(N_CORES)))

    acc = np.zeros((O, B), dtype=np.float32)
    for c in range(N_CORES):
        part = res.results[c]["outp"]
        acc += part[0]
        acc += part[1]
    out = np.ascontiguousarray(acc.T)
    if np.any(b2 != 0.0):
        # per-expert output bias enters as gate @ b2 ([B,E] @ [E,O])
        gate = X @ gate_W + gate_b[None, :]
        gate -= gate.max(axis=1, keepdims=True)
        np.exp(gate, out=gate)
        gate /= gate.sum(axis=1, keepdims=True)
        out += gate @ b2
    return np.ascontiguousarray(out, dtype=np.float32)


if __name__ == "__main__":
    rng = np.random.default_rng(0)
    s = 0.02
    inputs = {
        "pooled_output": rng.standard_normal((B, H), dtype=np.float32),
        "gate_W": rng.standard_normal((H, E), dtype=np.float32) * s,
        "gate_b": np.zeros((E,), np.float32),
        "W1": rng.standard_normal((E, H, H), dtype=np.float32) * s,
        "b1": np.zeros((E, H), np.float32),
        "ln_g": np.ones((E, H), np.float32),
        "ln_b": np.zeros((E, H), np.float32),
        "W2": rng.standard_normal((E, H, O), dtype=np.float32) * s,
        "b2": np.zeros((E, O), np.float32),
    }
    out = kernel(**inputs)
    print("out", out.shape, out.dtype, np.abs(out).max())
